# revision 19
# baseline (speedup 1.0000x reference)
"""BasisAttention TRN2 kernel.

Data-parallel over batch: 16 batches / 8 cores = 2 per core. Each core:
  En = rmsnorm(E)            (norm weights folded into Wk/Wq)
  KmT = Wk' @ En.T           -> DRAM scratch, streamed back per n-tile
  per batch: Xn -> QT = Wq' @ Xn.T ; scoresT tiles = KmT.T @ QT ;
             gs = max over s ; top-64 cutoff (max8/match_replace ladder) ;
             dense/sparse softmax blend -> weights
  O = rmsnorm((weights @ En) * sn_w) * on_w

Matmuls on the score path run in full fp32 (PE 4-pass) so the top-64
selection is exact vs the fp32 reference; the final small matmul runs fp32r.

n-ordering: embedding n lives at (partition p, tile t) with n = p*32 + t
everywhere (E loaded with stride-32 row interleave), which makes the
weights DMA and the final matmul layouts contiguous.
"""

from contextlib import ExitStack

import numpy as np

import concourse.bass as bass
import concourse.bass_isa as bass_isa
import concourse.mybir as mybir
import concourse.tile as tile
from concourse import bacc
from concourse.bass_utils import run_bass_kernel_spmd
from concourse.masks import make_identity

F32 = mybir.dt.float32
F32R = mybir.dt.float32r
AX = mybir.AxisListType.X
ALU = mybir.AluOpType
ACT = mybir.ActivationFunctionType

B, S, D, N, KTOP = 16, 2048, 1024, 4096, 64
NCORES = 8
BPC = B // NCORES          # batches per core
P = 128
ET = D // P                # 8 e tiles
DT = D // P                # 8 d tiles
NT = N // P                # 32 n tiles
ST = S // P                # 16 s tiles per batch
SC = S // 512              # 4 s chunks of 512
EPS = 1e-6
NEG = -1.0e30

# top-k ladder: gs reshaped to [8, 512]; stage1 extracts top-64 per row,
# stage2 extracts global top-64 from the 512 candidates.
R1, R2 = 8, 8


def _rmsnorm_rows(nc, pool_small, scratch, x, out):
    """out = x * rsqrt(mean(x^2, free) + eps). x, out: [p, F]."""
    p, f = x.shape[0], x.shape[-1]
    ssq = pool_small.tile([p, 1], F32, tag="ssq")
    nc.scalar.activation(scratch, x, ACT.Square, accum_out=ssq)
    ms = pool_small.tile([p, 1], F32, tag="ms")
    nc.vector.tensor_scalar(ms, ssq, 1.0 / f, EPS, ALU.mult, ALU.add)
    sq = pool_small.tile([p, 1], F32, tag="sq")
    nc.scalar.sqrt(sq, ms)
    rs = pool_small.tile([p, 1], F32, tag="rs")
    nc.vector.reciprocal(rs, sq)
    nc.scalar.activation(out, x, ACT.Copy, scale=rs)


def emit(nc, tc, io, ctx):
    ident_pool = ctx.enter_context(tc.tile_pool(name="ident", bufs=1))
    persist = ctx.enter_context(tc.tile_pool(name="persist", bufs=1))
    small = ctx.enter_context(tc.tile_pool(name="small", bufs=4))
    dram = ctx.enter_context(tc.tile_pool(name="dram", bufs=1, space="DRAM"))

    ident = ident_pool.tile([P, P], F32)
    make_identity(nc, ident)

    # --- constants / norm weights ---
    # xn_sb[p, et] = xn_w[et*128 + p]; same for sn
    xn_sb = persist.tile([P, ET], F32, tag="xn_sb")
    sn_sb = persist.tile([P, ET], F32, tag="sn_sb")
    nc.sync.dma_start(xn_sb, io["xn_w"].rearrange("1 (t p) -> p t", p=P))
    nc.sync.dma_start(sn_sb, io["sn_w"].rearrange("1 (t p) -> p t", p=P))
    sn2 = persist.tile([BPC, D], F32, tag="sn2")
    on2 = persist.tile([BPC, D], F32, tag="on2")
    a11 = persist.tile([1, 1], F32, tag="a11")
    nc.sync.dma_start(a11, io["alpha"])
    alpha_sb = persist.tile([P, 1], F32, tag="alpha_sb")
    nc.gpsimd.partition_broadcast(alpha_sb, a11)
    sn1 = persist.tile([1, D], F32, tag="sn1")
    on1 = persist.tile([1, D], F32, tag="on1")
    nc.sync.dma_start(sn1, io["sn_w"])
    nc.sync.dma_start(on1, io["on_w"])
    nc.gpsimd.partition_broadcast(sn2, sn1, channels=BPC)
    nc.gpsimd.partition_broadcast(on2, on1, channels=BPC)

    # persistent big tiles
    wqT = persist.tile([P, ET, D], F32, tag="wqT")    # (e_sub, et, d) = Wq[d,e]*xn_w[e]
    gs = [persist.tile([P, NT], F32, tag=f"gs{b}", name=f"gs{b}") for b in range(BPC)]
    wgt = [persist.tile([P, NT], F32, tag=f"wgt{b}", name=f"wgt{b}") for b in range(BPC)]
    wpair = persist.tile([P, BPC, NT], F32, tag="wpair")

    # DRAM scratch
    eh_dram = dram.tile([N, D], F32, tag="eh_dram")           # normalized E (no sn_w)
    kmT_dram = dram.tile([NT, P, DT, P], F32, tag="kmT_dram")  # (t, d_sub, dt, n_sub)

    # --- phases W + E (wkT lives only here) ---
    e_perm = io["E"].rearrange("(p t) e -> t p e", t=NT)
    eh_perm = eh_dram.rearrange("(p t) e -> t p e", t=NT)
    with tc.tile_pool(name="wk_pool", bufs=1) as wk_pool:
        wkT = wk_pool.tile([P, ET, D], F32, tag="wkT")   # (e_sub, et, d) = Wk[d,e]*sn_w[e]

        # phase W: transpose + scale Wq, Wk
        with tc.tile_pool(name="wload", bufs=2) as wload, \
             tc.tile_pool(name="ptr", bufs=2, space="PSUM") as ptr:
            for (w_in, w_out, w_scale) in ((io["Wq"], wqT, xn_sb), (io["Wk"], wkT, sn_sb)):
                for dt in range(DT):
                    w_nat = wload.tile([P, D], F32, tag="w_nat")
                    nc.sync.dma_start(w_nat, w_in[dt * P:(dt + 1) * P, :])
                    for eg in range(2):
                        ps = ptr.tile([P, 512], F32, tag="ps")
                        for i in range(4):
                            et = eg * 4 + i
                            nc.tensor.transpose(ps[:, i * P:(i + 1) * P],
                                                w_nat[:, et * P:(et + 1) * P], ident)
                        for i in range(4):
                            et = eg * 4 + i
                            nc.vector.tensor_scalar_mul(
                                w_out[:, et, dt * P:(dt + 1) * P],
                                ps[:, i * P:(i + 1) * P],
                                w_scale[:, et:et + 1])

        # phase E: normalize E, spill Eh, build KmT -> DRAM
        with tc.tile_pool(name="eload", bufs=3) as eload, \
             tc.tile_pool(name="escr", bufs=2) as escr, \
             tc.tile_pool(name="ehT", bufs=2) as ehT_pool, \
             tc.tile_pool(name="kst", bufs=3) as kst, \
             tc.tile_pool(name="ptr", bufs=2, space="PSUM") as ptr, \
             tc.tile_pool(name="pmm", bufs=3, space="PSUM") as pmm:
            for tg in range(NT // 4):       # n-chunks of 4 tiles = 512 columns
                ehT = ehT_pool.tile([P, ET, 512], F32, tag="ehT")
                for i4 in range(4):
                    t = tg * 4 + i4
                    e_nat = eload.tile([P, D], F32, tag="e_nat")
                    nc.sync.dma_start(e_nat, e_perm[t])
                    scr = escr.tile([P, D], F32, tag="scr")
                    eh = eload.tile([P, D], F32, tag="eh")
                    _rmsnorm_rows(nc, small, scr, e_nat, eh)
                    nc.sync.dma_start(eh_perm[t], eh)
                    for eg in range(2):
                        ps = ptr.tile([P, 512], F32, tag="ps")
                        for i in range(4):
                            et = eg * 4 + i
                            nc.tensor.transpose(ps[:, i * P:(i + 1) * P],
                                                eh[:, et * P:(et + 1) * P], ident)
                        nc.vector.tensor_copy(
                            ehT[:, eg * 4:(eg + 1) * 4, i4 * P:(i4 + 1) * P], ps)
                for dt in range(DT):
                    psk = pmm.tile([P, 512], F32, tag="psk")
                    for et in range(ET):
                        nc.tensor.matmul(psk, wkT[:, et, dt * P:(dt + 1) * P],
                                         ehT[:, et, :], start=(et == 0), stop=(et == ET - 1))
                    ks = kst.tile([P, 512], F32, tag="ks")
                    nc.vector.tensor_copy(ks, psk)
                    for i4 in range(4):
                        nc.sync.dma_start(kmT_dram[tg * 4 + i4, :, dt, :],
                                          ks[:, i4 * P:(i4 + 1) * P])

    # --- per batch: Q build, scores, topk, softmax ---
    for b in range(BPC):
      with tc.tile_pool(name="qT_pool", bufs=1) as qT_pool:
        qT = qT_pool.tile([P, DT, S], F32, tag="qT")    # (d_sub, dt, s)
        with tc.tile_pool(name="xload", bufs=3) as xload, \
             tc.tile_pool(name="xscr", bufs=2) as xscr, \
             tc.tile_pool(name="xnT", bufs=2) as xnT_pool, \
             tc.tile_pool(name="ptr", bufs=2, space="PSUM") as ptr, \
             tc.tile_pool(name="pmm", bufs=3, space="PSUM") as pmm:
            for sc in range(SC):
                xnT = xnT_pool.tile([P, ET, 512], F32, tag="xnT")
                for s4 in range(4):
                    st = sc * 4 + s4
                    x_nat = xload.tile([P, D], F32, tag="x_nat")
                    nc.sync.dma_start(x_nat, io["X"][b, st * P:(st + 1) * P, :])
                    scr = xscr.tile([P, D], F32, tag="scr")
                    xn = xload.tile([P, D], F32, tag="xn")
                    _rmsnorm_rows(nc, small, scr, x_nat, xn)
                    for eg in range(2):
                        ps = ptr.tile([P, 512], F32, tag="ps")
                        for i in range(4):
                            et = eg * 4 + i
                            nc.tensor.transpose(ps[:, i * P:(i + 1) * P],
                                                xn[:, et * P:(et + 1) * P], ident)
                        nc.vector.tensor_copy(
                            xnT[:, eg * 4:(eg + 1) * 4, s4 * P:(s4 + 1) * P], ps)
                for dt in range(DT):
                    psq = pmm.tile([P, 512], F32, tag="psq")
                    for et in range(ET):
                        nc.tensor.matmul(psq, wqT[:, et, dt * P:(dt + 1) * P],
                                         xnT[:, et, :], start=(et == 0), stop=(et == ET - 1))
                    nc.vector.tensor_copy(qT[:, dt, sc * 512:(sc + 1) * 512], psq)

        # scores: stream KmT tiles, accumulate gs = max over s
        with tc.tile_pool(name="ktile", bufs=3) as ktile, \
             tc.tile_pool(name="red", bufs=3) as redp, \
             tc.tile_pool(name="pmm", bufs=3, space="PSUM") as pmm:
            for t in range(NT):
                kt = ktile.tile([P, DT, P], F32, tag="kt")
                nc.sync.dma_start(kt, kmT_dram[t])
                red = redp.tile([P, SC], F32, tag="red")
                for sc in range(SC):
                    pss = pmm.tile([P, 512], F32, tag="pss")
                    for dt in range(DT):
                        nc.tensor.matmul(pss, kt[:, dt, :],
                                         qT[:, dt, sc * 512:(sc + 1) * 512],
                                         start=(dt == 0), stop=(dt == DT - 1))
                    nc.vector.reduce_max(red[:, sc:sc + 1], pss, axis=AX)
                nc.vector.reduce_max(gs[b][:, t:t + 1], red, axis=AX)
            # scores scale factor hidden^-0.5
            nc.vector.tensor_scalar_mul(gs[b], gs[b], float(D) ** -0.5)

        # top-64 cutoff: values-only ladder on [8, 512] then [1, 512]
        with tc.tile_pool(name="tk", bufs=1) as tk:
            g8a = tk.tile([8, 512], F32, tag="g8a")
            g8b = tk.tile([8, 512], F32, tag="g8b")
            nc.sync.dma_start(g8a, gs[b])
            cand1 = tk.tile([8, R1 * 8], F32, tag="cand1")
            cur, nxt = g8a, g8b
            for r in range(R1):
                nc.vector.max(cand1[:, r * 8:(r + 1) * 8], cur)
                if r < R1 - 1:
                    nc.vector.match_replace(nxt, cand1[:, r * 8:(r + 1) * 8], cur, NEG)
                    cur, nxt = nxt, cur
            c1r = tk.tile([1, 8 * R1 * 8], F32, tag="c1r")
            nc.sync.dma_start(c1r, cand1)
            c2a = tk.tile([1, 8 * R1 * 8], F32, tag="c2a")
            cand2 = tk.tile([1, R2 * 8], F32, tag="cand2")
            cur, nxt = c1r, c2a
            for r in range(R2):
                nc.vector.max(cand2[:, r * 8:(r + 1) * 8], cur)
                if r < R2 - 1:
                    nc.vector.match_replace(nxt, cand2[:, r * 8:(r + 1) * 8], cur, NEG)
                    cur, nxt = nxt, cur
            cm = tk.tile([1, 2], F32, tag="cm")
            nc.vector.tensor_copy(cm[:, 0:1], cand2[:, 0:1])
            nc.vector.tensor_copy(cm[:, 1:2], cand2[:, KTOP - 1:KTOP])
            bc = tk.tile([P, 2], F32, tag="bc")
            nc.gpsimd.partition_broadcast(bc, cm)

            # softmax: dense over all, sparse over mask(gs >= cutoff)
            nbm = tk.tile([P, 1], F32, tag="nbm")
            nc.vector.tensor_scalar_mul(nbm, bc[:, 0:1], -1.0)
            expd = tk.tile([P, NT], F32, tag="expd")
            nc.scalar.activation(expd, gs[b], ACT.Exp, bias=nbm)
            mask01 = tk.tile([P, NT], F32, tag="mask01")
            nc.vector.tensor_scalar(mask01, gs[b], bc[:, 1:2], None, ALU.is_ge)
            exps = tk.tile([P, NT], F32, tag="exps")
            nc.vector.tensor_mul(exps, expd, mask01)
            sums = tk.tile([P, 2], F32, tag="sums")
            nc.vector.reduce_sum(sums[:, 0:1], expd, axis=AX)
            nc.vector.reduce_sum(sums[:, 1:2], exps, axis=AX)
            sumr = tk.tile([P, 2], F32, tag="sumr")
            nc.gpsimd.partition_all_reduce(sumr, sums, channels=P,
                                           reduce_op=bass_isa.ReduceOp.add)
            rcp = tk.tile([P, 2], F32, tag="rcp")
            nc.vector.reciprocal(rcp, sumr)
            dw = tk.tile([P, NT], F32, tag="dw")
            sw = tk.tile([P, NT], F32, tag="sw")
            nc.vector.tensor_scalar_mul(dw, expd, rcp[:, 0:1])
            nc.vector.tensor_scalar_mul(sw, exps, rcp[:, 1:2])
            diff = tk.tile([P, NT], F32, tag="diff")
            nc.vector.tensor_sub(diff, dw, sw)
            nc.vector.scalar_tensor_tensor(wgt[b], diff, alpha_sb, sw,
                                           ALU.mult, ALU.add)
            nc.sync.dma_start(io["W_part"][b].rearrange("(p t) -> p t", t=NT), wgt[b])
            nc.vector.tensor_copy(wpair[:, b, :], wgt[b])

    # --- final: O = rmsnorm((w @ Eh) * sn_w) * on_w ---
    with tc.tile_pool(name="fload", bufs=3) as fload, \
         tc.tile_pool(name="fps", bufs=1, space="PSUM") as fps, \
         tc.tile_pool(name="fsb", bufs=1) as fsb:
        psz = [fps.tile([BPC, 512], F32, tag=f"psz{ec}", name=f"psz{ec}") for ec in range(2)]
        for tau in range(NT):
            ehm = fload.tile([P, D], F32, tag="ehm")
            nc.sync.dma_start(ehm, eh_perm[tau])
            wp = wpair[:, :, tau]
            for ec in range(2):
                nc.tensor.matmul(psz[ec], wp,
                                 ehm[:, ec * 512:(ec + 1) * 512],
                                 start=(tau == 0), stop=(tau == NT - 1))
        zsb = fsb.tile([BPC, D], F32, tag="zsb")
        for ec in range(2):
            nc.vector.tensor_mul(zsb[:, ec * 512:(ec + 1) * 512], psz[ec],
                                 sn2[:, ec * 512:(ec + 1) * 512])
        scr = fsb.tile([BPC, D], F32, tag="fscr")
        o2 = fsb.tile([BPC, D], F32, tag="o2")
        _rmsnorm_rows(nc, small, scr, zsb, o2)
        o3 = fsb.tile([BPC, D], F32, tag="o3")
        nc.vector.tensor_mul(o3, o2, on2)
        nc.sync.dma_start(io["O_part"], o3)




def build_program():
    nc = bacc.Bacc(trn_type="TRN2", target_bir_lowering=False, debug=False)
    io = {
        "X": nc.dram_tensor("X", [BPC, S, D], F32, kind="ExternalInput").ap(),
        "alpha": nc.dram_tensor("alpha", [1, 1], F32, kind="ExternalInput").ap(),
        "E": nc.dram_tensor("E", [N, D], F32, kind="ExternalInput").ap(),
        "Wq": nc.dram_tensor("Wq", [D, D], F32, kind="ExternalInput").ap(),
        "Wk": nc.dram_tensor("Wk", [D, D], F32, kind="ExternalInput").ap(),
        "xn_w": nc.dram_tensor("xn_w", [1, D], F32, kind="ExternalInput").ap(),
        "sn_w": nc.dram_tensor("sn_w", [1, D], F32, kind="ExternalInput").ap(),
        "on_w": nc.dram_tensor("on_w", [1, D], F32, kind="ExternalInput").ap(),
        "O_part": nc.dram_tensor("O_part", [BPC, D], F32, kind="ExternalOutput").ap(),
        "W_part": nc.dram_tensor("W_part", [BPC, N], F32, kind="ExternalOutput").ap(),
    }
    with tile.TileContext(nc) as tc:
        with ExitStack() as ctx:
            emit(nc, tc, io, ctx)
    nc.compile()
    return nc


_NC_CACHE = None


def kernel(X, alpha, E, Wq, Wk, xn_w, sn_w, on_w):
    global _NC_CACHE
    if _NC_CACHE is None:
        _NC_CACHE = build_program()
    nc = _NC_CACHE
    f32 = lambda a: np.ascontiguousarray(np.asarray(a, dtype=np.float32))
    in_maps = []
    for c in range(NCORES):
        in_maps.append({
            "X": f32(X[BPC * c:BPC * (c + 1)]),
            "alpha": f32(alpha).reshape(1, 1),
            "E": f32(E), "Wq": f32(Wq), "Wk": f32(Wk),
            "xn_w": f32(xn_w).reshape(1, D), "sn_w": f32(sn_w).reshape(1, D),
            "on_w": f32(on_w).reshape(1, D),
        })
    res = run_bass_kernel_spmd(nc, in_maps, list(range(NCORES)))
    O = np.concatenate([res.results[c]["O_part"] for c in range(NCORES)], axis=0)
    W = np.concatenate([res.results[c]["W_part"] for c in range(NCORES)], axis=0)
    return O, W


# revision 47
# speedup vs baseline: 8.0041x; 8.0041x over previous
"""BasisAttention TRN2 kernel.

Data-parallel over batch: 16 batches / 8 cores = 2 per core. Each core:
  En = rmsnorm(E)            (norm weights folded into Wk/Wq)
  KmT = Wk' @ En.T           -> DRAM scratch, streamed back per n-tile
  per batch: Xn -> QT = Wq' @ Xn.T ; scoresT tiles = KmT.T @ QT ;
             gs = max over s ; top-64 cutoff (max8/match_replace ladder) ;
             dense/sparse softmax blend -> weights
  O = rmsnorm((weights @ En) * sn_w) * on_w

Matmuls on the score path run in full fp32 (PE 4-pass) so the top-64
selection is exact vs the fp32 reference; the final small matmul runs fp32r.

n-ordering: embedding n lives at (partition p, tile t) with n = p*32 + t
everywhere (E loaded with stride-32 row interleave), which makes the
weights DMA and the final matmul layouts contiguous.
"""

from contextlib import ExitStack

import numpy as np

import concourse.bass as bass
import concourse.bass_isa as bass_isa
import concourse.mybir as mybir
import concourse.tile as tile
from concourse import bacc
from concourse.bass_utils import run_bass_kernel_spmd
from concourse.masks import make_identity

F32 = mybir.dt.float32
F32R = mybir.dt.float32r
AX = mybir.AxisListType.X
ALU = mybir.AluOpType
ACT = mybir.ActivationFunctionType

B, S, D, N, KTOP = 16, 2048, 1024, 4096, 64
NCORES = 8
BPC = B // NCORES          # batches per core
P = 128
ET = D // P                # 8 e tiles
DT = D // P                # 8 d tiles
NT = N // P                # 32 n tiles
ST = S // P                # 16 s tiles per batch
SC = S // 512              # 4 s chunks of 512
EPS = 1e-6
NEG = -1.0e30

# top-k ladder: gs reshaped to [8, 512]; stage1 extracts top-64 per row,
# stage2 extracts global top-64 from the 512 candidates.
R1, R2 = 8, 8


def _rmsnorm_rows(nc, pool_small, scratch, x, out):
    """out = x * rsqrt(mean(x^2, free) + eps). x, out: [p, F]."""
    p, f = x.shape[0], x.shape[-1]
    ssq = pool_small.tile([p, 1], F32, tag="ssq")
    nc.scalar.activation(scratch, x, ACT.Square, accum_out=ssq)
    ms = pool_small.tile([p, 1], F32, tag="ms")
    nc.vector.tensor_scalar(ms, ssq, 1.0 / f, EPS, ALU.mult, ALU.add)
    sq = pool_small.tile([p, 1], F32, tag="sq")
    nc.scalar.sqrt(sq, ms)
    rs = pool_small.tile([p, 1], F32, tag="rs")
    nc.vector.reciprocal(rs, sq)
    nc.scalar.activation(out, x, ACT.Copy, scale=rs)


def emit(nc, tc, io, ctx):
    ident_pool = ctx.enter_context(tc.tile_pool(name="ident", bufs=1))
    persist = ctx.enter_context(tc.tile_pool(name="persist", bufs=1))
    small = ctx.enter_context(tc.tile_pool(name="small", bufs=4))
    dram = ctx.enter_context(tc.tile_pool(name="dram", bufs=1, space="DRAM"))

    ident = ident_pool.tile([P, P], F32)
    make_identity(nc, ident)

    # --- constants / norm weights ---
    # xn_sb[p, et] = xn_w[et*128 + p]; same for sn
    xn_sb = persist.tile([P, ET], F32, tag="xn_sb")
    sn_sb = persist.tile([P, ET], F32, tag="sn_sb")
    nc.sync.dma_start(xn_sb, io["xn_w"].rearrange("1 (t p) -> p t", p=P))
    nc.sync.dma_start(sn_sb, io["sn_w"].rearrange("1 (t p) -> p t", p=P))
    sn2 = persist.tile([BPC, D], F32, tag="sn2")
    on2 = persist.tile([BPC, D], F32, tag="on2")
    a11 = persist.tile([1, 1], F32, tag="a11")
    nc.sync.dma_start(a11, io["alpha"])
    alpha_sb = persist.tile([P, 1], F32, tag="alpha_sb")
    nc.gpsimd.partition_broadcast(alpha_sb, a11)
    sn1 = persist.tile([1, D], F32, tag="sn1")
    on1 = persist.tile([1, D], F32, tag="on1")
    nc.sync.dma_start(sn1, io["sn_w"])
    nc.sync.dma_start(on1, io["on_w"])
    nc.gpsimd.partition_broadcast(sn2, sn1, channels=BPC)
    nc.gpsimd.partition_broadcast(on2, on1, channels=BPC)

    # persistent big tiles
    wqT = persist.tile([P, ET, D], F32, tag="wqT")    # (e_sub, et, d) = Wq[d,e]*xn_w[e]
    gs = [persist.tile([P, NT], F32, tag=f"gs{b}", name=f"gs{b}") for b in range(BPC)]
    wgt = [persist.tile([P, NT], F32, tag=f"wgt{b}", name=f"wgt{b}") for b in range(BPC)]
    wpair = persist.tile([P, BPC, NT], F32, tag="wpair")

    # DRAM scratch
    eh_dram = dram.tile([N, D], F32, tag="eh_dram")           # normalized E (no sn_w)
    kmT_dram = dram.tile([NT, P, DT, P], F32, tag="kmT_dram")  # (t, d_sub, dt, n_sub)

    # --- phases W + E (wkT lives only here) ---
    e_perm = io["E"].rearrange("(p t) e -> t p e", t=NT)
    eh_perm = eh_dram.rearrange("(p t) e -> t p e", t=NT)
    with tc.tile_pool(name="wk_pool", bufs=1) as wk_pool:
        wkT = wk_pool.tile([P, ET, D], F32, tag="wkT")   # (e_sub, et, d) = Wk[d,e]*sn_w[e]

        # phase W: transpose + scale Wq, Wk
        with tc.tile_pool(name="wload", bufs=2) as wload, \
             tc.tile_pool(name="ptr", bufs=2, space="PSUM") as ptr:
            for (w_in, w_out, w_scale) in ((io["Wq"], wqT, xn_sb), (io["Wk"], wkT, sn_sb)):
                for dt in range(DT):
                    w_nat = wload.tile([P, D], F32, tag="w_nat")
                    nc.sync.dma_start(w_nat, w_in[dt * P:(dt + 1) * P, :])
                    for eg in range(2):
                        ps = ptr.tile([P, 512], F32, tag="ps")
                        for i in range(4):
                            et = eg * 4 + i
                            nc.tensor.transpose(ps[:, i * P:(i + 1) * P],
                                                w_nat[:, et * P:(et + 1) * P], ident)
                        for i in range(4):
                            et = eg * 4 + i
                            nc.vector.tensor_scalar_mul(
                                w_out[:, et, dt * P:(dt + 1) * P],
                                ps[:, i * P:(i + 1) * P],
                                w_scale[:, et:et + 1])

        # phase E: normalize E, spill Eh, build KmT -> DRAM
        with tc.tile_pool(name="eload", bufs=3) as eload, \
             tc.tile_pool(name="escr", bufs=2) as escr, \
             tc.tile_pool(name="ehT", bufs=2) as ehT_pool, \
             tc.tile_pool(name="kst", bufs=3) as kst, \
             tc.tile_pool(name="ptr", bufs=2, space="PSUM") as ptr, \
             tc.tile_pool(name="pmm", bufs=3, space="PSUM") as pmm:
            for tg in range(NT // 4):       # n-chunks of 4 tiles = 512 columns
                ehT = ehT_pool.tile([P, ET, 512], F32, tag="ehT")
                for i4 in range(4):
                    t = tg * 4 + i4
                    e_nat = eload.tile([P, D], F32, tag="e_nat")
                    nc.sync.dma_start(e_nat, e_perm[t])
                    scr = escr.tile([P, D], F32, tag="scr")
                    eh = eload.tile([P, D], F32, tag="eh")
                    _rmsnorm_rows(nc, small, scr, e_nat, eh)
                    nc.sync.dma_start(eh_perm[t], eh)
                    for eg in range(2):
                        ps = ptr.tile([P, 512], F32, tag="ps")
                        for i in range(4):
                            et = eg * 4 + i
                            nc.tensor.transpose(ps[:, i * P:(i + 1) * P],
                                                eh[:, et * P:(et + 1) * P], ident)
                        nc.vector.tensor_copy(
                            ehT[:, eg * 4:(eg + 1) * 4, i4 * P:(i4 + 1) * P], ps)
                for dt in range(DT):
                    psk = pmm.tile([P, 512], F32, tag="psk")
                    for et in range(ET):
                        nc.tensor.matmul(psk, wkT[:, et, dt * P:(dt + 1) * P],
                                         ehT[:, et, :], start=(et == 0), stop=(et == ET - 1))
                    ks = kst.tile([P, 512], F32, tag="ks")
                    nc.vector.tensor_copy(ks, psk)
                    for i4 in range(4):
                        nc.sync.dma_start(kmT_dram[tg * 4 + i4, :, dt, :],
                                          ks[:, i4 * P:(i4 + 1) * P])

    # --- per batch: Q build, scores, topk, softmax ---
    for b in range(BPC):
      with tc.tile_pool(name="qT_pool", bufs=1) as qT_pool:
        qT = qT_pool.tile([P, DT, S], F32, tag="qT")    # (d_sub, dt, s)
        with tc.tile_pool(name="xload", bufs=3) as xload, \
             tc.tile_pool(name="xscr", bufs=2) as xscr, \
             tc.tile_pool(name="xnT", bufs=2) as xnT_pool, \
             tc.tile_pool(name="ptr", bufs=2, space="PSUM") as ptr, \
             tc.tile_pool(name="pmm", bufs=3, space="PSUM") as pmm:
            for sc in range(SC):
                xnT = xnT_pool.tile([P, ET, 512], F32, tag="xnT")
                for s4 in range(4):
                    st = sc * 4 + s4
                    x_nat = xload.tile([P, D], F32, tag="x_nat")
                    nc.sync.dma_start(x_nat, io["X"][b, st * P:(st + 1) * P, :])
                    scr = xscr.tile([P, D], F32, tag="scr")
                    xn = xload.tile([P, D], F32, tag="xn")
                    _rmsnorm_rows(nc, small, scr, x_nat, xn)
                    for eg in range(2):
                        ps = ptr.tile([P, 512], F32, tag="ps")
                        for i in range(4):
                            et = eg * 4 + i
                            nc.tensor.transpose(ps[:, i * P:(i + 1) * P],
                                                xn[:, et * P:(et + 1) * P], ident)
                        nc.vector.tensor_copy(
                            xnT[:, eg * 4:(eg + 1) * 4, s4 * P:(s4 + 1) * P], ps)
                for dt in range(DT):
                    psq = pmm.tile([P, 512], F32, tag="psq")
                    for et in range(ET):
                        nc.tensor.matmul(psq, wqT[:, et, dt * P:(dt + 1) * P],
                                         xnT[:, et, :], start=(et == 0), stop=(et == ET - 1))
                    nc.vector.tensor_copy(qT[:, dt, sc * 512:(sc + 1) * 512], psq)

        # scores: stream KmT tiles, accumulate gs = max over s
        with tc.tile_pool(name="ktile", bufs=3) as ktile, \
             tc.tile_pool(name="red", bufs=3) as redp, \
             tc.tile_pool(name="pmm", bufs=3, space="PSUM") as pmm:
            for t in range(NT):
                kt = ktile.tile([P, DT, P], F32, tag="kt")
                nc.sync.dma_start(kt, kmT_dram[t])
                red = redp.tile([P, SC], F32, tag="red")
                for sc in range(SC):
                    pss = pmm.tile([P, 512], F32, tag="pss")
                    for dt in range(DT):
                        nc.tensor.matmul(pss, kt[:, dt, :],
                                         qT[:, dt, sc * 512:(sc + 1) * 512],
                                         start=(dt == 0), stop=(dt == DT - 1))
                    nc.vector.reduce_max(red[:, sc:sc + 1], pss, axis=AX)
                nc.vector.reduce_max(gs[b][:, t:t + 1], red, axis=AX)
            # scores scale factor hidden^-0.5
            nc.vector.tensor_scalar_mul(gs[b], gs[b], float(D) ** -0.5)

        # top-64 cutoff: values-only ladder on [8, 512] then [1, 512]
        with tc.tile_pool(name="tk", bufs=1) as tk:
            g8a = tk.tile([8, 512], F32, tag="g8a")
            g8b = tk.tile([8, 512], F32, tag="g8b")
            nc.sync.dma_start(g8a, gs[b])
            cand1 = tk.tile([8, R1 * 8], F32, tag="cand1")
            cur, nxt = g8a, g8b
            for r in range(R1):
                nc.vector.max(cand1[:, r * 8:(r + 1) * 8], cur)
                if r < R1 - 1:
                    nc.vector.match_replace(nxt, cand1[:, r * 8:(r + 1) * 8], cur, NEG)
                    cur, nxt = nxt, cur
            c1r = tk.tile([1, 8 * R1 * 8], F32, tag="c1r")
            nc.sync.dma_start(c1r, cand1)
            c2a = tk.tile([1, 8 * R1 * 8], F32, tag="c2a")
            cand2 = tk.tile([1, R2 * 8], F32, tag="cand2")
            cur, nxt = c1r, c2a
            for r in range(R2):
                nc.vector.max(cand2[:, r * 8:(r + 1) * 8], cur)
                if r < R2 - 1:
                    nc.vector.match_replace(nxt, cand2[:, r * 8:(r + 1) * 8], cur, NEG)
                    cur, nxt = nxt, cur
            cm = tk.tile([1, 2], F32, tag="cm")
            nc.vector.tensor_copy(cm[:, 0:1], cand2[:, 0:1])
            nc.vector.tensor_copy(cm[:, 1:2], cand2[:, KTOP - 1:KTOP])
            bc = tk.tile([P, 2], F32, tag="bc")
            nc.gpsimd.partition_broadcast(bc, cm)

            # softmax: dense over all, sparse over mask(gs >= cutoff)
            nbm = tk.tile([P, 1], F32, tag="nbm")
            nc.vector.tensor_scalar_mul(nbm, bc[:, 0:1], -1.0)
            expd = tk.tile([P, NT], F32, tag="expd")
            nc.scalar.activation(expd, gs[b], ACT.Exp, bias=nbm)
            mask01 = tk.tile([P, NT], F32, tag="mask01")
            nc.vector.tensor_scalar(mask01, gs[b], bc[:, 1:2], None, ALU.is_ge)
            exps = tk.tile([P, NT], F32, tag="exps")
            nc.vector.tensor_mul(exps, expd, mask01)
            sums = tk.tile([P, 2], F32, tag="sums")
            nc.vector.reduce_sum(sums[:, 0:1], expd, axis=AX)
            nc.vector.reduce_sum(sums[:, 1:2], exps, axis=AX)
            sumr = tk.tile([P, 2], F32, tag="sumr")
            nc.gpsimd.partition_all_reduce(sumr, sums, channels=P,
                                           reduce_op=bass_isa.ReduceOp.add)
            rcp = tk.tile([P, 2], F32, tag="rcp")
            nc.vector.reciprocal(rcp, sumr)
            dw = tk.tile([P, NT], F32, tag="dw")
            sw = tk.tile([P, NT], F32, tag="sw")
            nc.vector.tensor_scalar_mul(dw, expd, rcp[:, 0:1])
            nc.vector.tensor_scalar_mul(sw, exps, rcp[:, 1:2])
            diff = tk.tile([P, NT], F32, tag="diff")
            nc.vector.tensor_sub(diff, dw, sw)
            nc.vector.scalar_tensor_tensor(wgt[b], diff, alpha_sb, sw,
                                           ALU.mult, ALU.add)
            nc.sync.dma_start(io["W_part"][b].rearrange("(p t) -> p t", t=NT), wgt[b])
            nc.vector.tensor_copy(wpair[:, b, :], wgt[b])

    # --- final: O = rmsnorm((w @ Eh) * sn_w) * on_w ---
    with tc.tile_pool(name="fload", bufs=3) as fload, \
         tc.tile_pool(name="fps", bufs=1, space="PSUM") as fps, \
         tc.tile_pool(name="fsb", bufs=1) as fsb:
        psz = [fps.tile([BPC, 512], F32, tag=f"psz{ec}", name=f"psz{ec}") for ec in range(2)]
        for tau in range(NT):
            ehm = fload.tile([P, D], F32, tag="ehm")
            nc.sync.dma_start(ehm, eh_perm[tau])
            wp = wpair[:, :, tau]
            for ec in range(2):
                nc.tensor.matmul(psz[ec], wp,
                                 ehm[:, ec * 512:(ec + 1) * 512],
                                 start=(tau == 0), stop=(tau == NT - 1))
        zsb = fsb.tile([BPC, D], F32, tag="zsb")
        for ec in range(2):
            nc.vector.tensor_mul(zsb[:, ec * 512:(ec + 1) * 512], psz[ec],
                                 sn2[:, ec * 512:(ec + 1) * 512])
        scr = fsb.tile([BPC, D], F32, tag="fscr")
        o2 = fsb.tile([BPC, D], F32, tag="o2")
        _rmsnorm_rows(nc, small, scr, zsb, o2)
        o3 = fsb.tile([BPC, D], F32, tag="o3")
        nc.vector.tensor_mul(o3, o2, on2)
        nc.sync.dma_start(io["O_part"], o3)




def build_program():
    nc = bacc.Bacc(trn_type="TRN2", target_bir_lowering=False, debug=False)
    io = {
        "X": nc.dram_tensor("X", [BPC, S, D], F32, kind="ExternalInput").ap(),
        "alpha": nc.dram_tensor("alpha", [1, 1], F32, kind="ExternalInput").ap(),
        "E": nc.dram_tensor("E", [N, D], F32, kind="ExternalInput").ap(),
        "Wq": nc.dram_tensor("Wq", [D, D], F32, kind="ExternalInput").ap(),
        "Wk": nc.dram_tensor("Wk", [D, D], F32, kind="ExternalInput").ap(),
        "xn_w": nc.dram_tensor("xn_w", [1, D], F32, kind="ExternalInput").ap(),
        "sn_w": nc.dram_tensor("sn_w", [1, D], F32, kind="ExternalInput").ap(),
        "on_w": nc.dram_tensor("on_w", [1, D], F32, kind="ExternalInput").ap(),
        "O_part": nc.dram_tensor("O_part", [BPC, D], F32, kind="ExternalOutput").ap(),
        "W_part": nc.dram_tensor("W_part", [BPC, N], F32, kind="ExternalOutput").ap(),
    }
    with tile.TileContext(nc) as tc:
        with ExitStack() as ctx:
            emit(nc, tc, io, ctx)
    nc.compile()
    return nc


_NC_CACHE = None


def kernel(X, alpha, E, Wq, Wk, xn_w, sn_w, on_w):
    global _NC_CACHE
    if _NC_CACHE is None:
        _NC_CACHE = build_program()
    nc = _NC_CACHE
    f32 = lambda a: np.ascontiguousarray(np.asarray(a, dtype=np.float32))
    in_maps = []
    for c in range(NCORES):
        in_maps.append({
            "X": f32(X[BPC * c:BPC * (c + 1)]),
            "alpha": f32(alpha).reshape(1, 1),
            "E": f32(E), "Wq": f32(Wq), "Wk": f32(Wk),
            "xn_w": f32(xn_w).reshape(1, D), "sn_w": f32(sn_w).reshape(1, D),
            "on_w": f32(on_w).reshape(1, D),
        })
    res = run_bass_kernel_spmd(nc, in_maps, list(range(NCORES)))
    O = np.concatenate([res.results[c]["O_part"] for c in range(NCORES)], axis=0)
    W = np.concatenate([res.results[c]["W_part"] for c in range(NCORES)], axis=0)
    return O, W
